# revision 28
# baseline (speedup 1.0000x reference)
"""Multi-head attention (B=2, S=2048, D=1024, H=16, d_k=64) on 8 trn2 cores.

Sharding: batch (2) x head-groups (4 groups of 4 heads). Each core computes
its batch's full sequence for its 4 heads plus the partial output projection
(w_o row-sharded); host sums the 4 partials per batch and adds b_o.

Schedule (v5): exp conveyor of 128 [128,1024] tiles (~1.2us cadence on the
scalar engine) starting ~30us in, right after the k projection and the
q pair-0 s-half-0 projection (streamed during the input DMA, balanced
across both HWDGE rings with few descriptors each so neither sequencer
blocks).  All other PE work (q s-half-1 / pair-1, v projection, part of
the output projection) drains into per-step conveyor slack via a budgeted
job queue.  Blocks are per-(head, s-half): PSUM = scores 2x[128,1024] +
one AV accumulator + 2 job banks.

Normalize (new): denominator row -> DVE reciprocal -> broadcast to 64
rows via a K=1 ones-column matmul in f32r (1 cycle/row) -> DVE multiply
straight out of PSUM into o2h.  No DRAM round-trip, no u-copy; the AV
accumulator handover is gated a few steps so the PE never waits.
"""

import numpy as np

P = 128
S = 2048
DM = 1024
DH = 256          # head dims per core (4 heads x 64)
H = 4             # heads per core
DK = 64
MC = DM // P      # 8 m-chunks
TC = S // P       # 16 t-chunks
ST = 1024         # s-tile width (conveyor block s-half)
N_CORES = 8

# conveyor block order: (head, st2). pair-0 heads first (q pair-1 is
# projected mid-conveyor); s0 blocks early so the s0 output projection can
# interleave before the conveyor ends.
ORDER = [(0, 0), (1, 0), (0, 1), (1, 1), (2, 0), (3, 0), (2, 1), (3, 1)]

CYC_PER_STEP = 2670   # PE-cycle budget per exp period
OPROJ_CONVEYOR = 7    # st7 chunks of the output projection done in-conveyor

_COMPILED = None


def _build():
    import concourse.bacc as bacc
    import concourse.mybir as mybir
    from concourse.tile import TileContext

    F32 = mybir.dt.float32
    F32R = mybir.dt.float32r
    BF16 = mybir.dt.bfloat16
    AF = mybir.ActivationFunctionType
    OP = mybir.AluOpType

    nc = bacc.Bacc(None, target_bir_lowering=False)

    xin = {}
    for t in ("q", "k", "v"):
        xin[t] = nc.dram_tensor(f"x{t}", [DM, S], BF16, kind="ExternalInput")
    wk = nc.dram_tensor("wk", [P, MC * DH], BF16, kind="ExternalInput")
    wq = nc.dram_tensor("wq", [P, MC * DH], BF16, kind="ExternalInput")
    # wv + wo merged; biases merged (fewer ring descriptors)
    wvo = nc.dram_tensor("wvo", [P, MC * DH + 2 * DM], BF16, kind="ExternalInput")
    ball = nc.dram_tensor("ball", [P, 2 + 2 + DH], F32, kind="ExternalInput")
    out = nc.dram_tensor("out", [S, DM], BF16, kind="ExternalOutput")

    with TileContext(nc) as tc:
        with (
            tc.tile_pool(name="persist", bufs=1) as pp,
            tc.tile_pool(name="xk", bufs=4) as xkp,
            tc.tile_pool(name="xv", bufs=2) as xvp,
            tc.tile_pool(name="xq", bufs=4) as xq,
            tc.tile_pool(name="athl", bufs=16) as hp,
            tc.tile_pool(name="norm", bufs=4) as xp,
            tc.tile_pool(name="oout", bufs=3) as op,
            tc.tile_pool(name="dram", bufs=4, space="DRAM") as dp,
            tc.tile_pool(name="ps_sc", bufs=2, space="PSUM") as ps_sc,
            tc.tile_pool(name="ps_av", bufs=1, space="PSUM") as ps_av,
            tc.tile_pool(name="ps_px", bufs=2, space="PSUM") as ps_px,
        ):
            qT = pp.tile([P, 2, S], BF16, name="qT")
            kT = pp.tile([P, 2, S], BF16, name="kT")
            vh = pp.tile([P, TC, H, DK + 1], BF16, name="vh")
            o2a = pp.tile([P, S], BF16, name="o2a")  # heads 0,1 normalized
            o2b = pp.tile([P, S], BF16, name="o2b")  # heads 2,3
            wk_sb = pp.tile([P, MC, DH], BF16, name="wk_sb")
            wq_sb = pp.tile([P, MC, DH], BF16, name="wq_sb")
            wvo_sb = pp.tile([P, MC * DH + 2 * DM], BF16, name="wvo_sb")
            ball_sb = pp.tile([P, 2 + 2 + DH], F32, name="ball_sb")

            wv_v = wvo_sb[:, 0 : MC * DH].rearrange("p (c n) -> p c n", c=MC)
            wo_v = wvo_sb[:, MC * DH :].rearrange("p (c n) -> p c n", c=2)
            bq_v = ball_sb[:, 0:2]
            bk_v = ball_sb[:, 2:4]
            bv_v = ball_sb[:, 4 : 4 + DH]

            nc.vector.memset(vh[:, :, :, DK : DK + 1], 1.0)

            # preload the exp spline table so the one-time ACT_TABLE_LOAD
            # doesn't sit inside the exp conveyor
            warm = op.tile([1, 2], F32, name="nt")
            nc.vector.memset(warm[0:1, :], 0.0)
            nc.scalar.activation(warm[0:1, 0:1], warm[0:1, 1:2], AF.Exp)

            # ---------------- input DMA ------------------------------------
            # Balanced two-ring split; the critical 7MB (w_k, w_q, x_k,
            # x_q s-half0) leads both rings.  x_q is s-half-major, x_v is
            # t-half-major ([P, 8, 1024] per half).
            # sync:   wk | k01 k23 | q(h0,side0) | v-h0 | q(h1,s0) q(h1,s1)
            # scalar: wq ball | k45 k67 | q(h0,side1) | v-h1 | wvo
            nc.sync.dma_start(wk_sb[:], wk[:].rearrange("p (c n) -> p c n", c=MC))
            nc.scalar.dma_start(wq_sb[:], wq[:].rearrange("p (c n) -> p c n", c=MC))
            nc.scalar.dma_start(ball_sb[:], ball[:])

            kg = []
            for g in range(4):
                x = xkp.tile([P, 2, S], BF16, name="xk")
                eng = nc.sync if g < 2 else nc.scalar
                eng.dma_start(
                    x[:],
                    xin["k"][2 * g * P : 2 * (g + 1) * P, :].rearrange(
                        "(c p) n -> p c n", p=P
                    ),
                )
                kg.append(x)
            xqg = {}
            for side in range(2):
                x = xq.tile([P, 4, ST], BF16, name="xqc")
                eng = nc.sync if side == 0 else nc.scalar
                eng.dma_start(
                    x[:],
                    xin["q"][4 * side * P : 4 * (side + 1) * P, 0:ST].rearrange(
                        "(c p) n -> p c n", p=P
                    ),
                )
                xqg[(0, side)] = x
            # x_v t-half-major: one [P, 8, 1024] tile per t-half
            vt = []
            for h in range(2):
                x = xvp.tile([P, MC, ST], BF16, name="xv")
                eng = nc.sync if h == 0 else nc.scalar
                eng.dma_start(
                    x[:],
                    xin["v"][:, h * ST : (h + 1) * ST].rearrange(
                        "(c p) n -> p c n", p=P
                    ),
                )
                vt.append(x)
            for side in range(2):
                x = xq.tile([P, 4, ST], BF16, name="xqc")
                nc.sync.dma_start(
                    x[:],
                    xin["q"][4 * side * P : 4 * (side + 1) * P, ST : 2 * ST].rearrange(
                        "(c p) n -> p c n", p=P
                    ),
                )
                xqg[(1, side)] = x
            nc.scalar.dma_start(wvo_sb[:], wvo[:])

            # ---------------- phase A: k (both pairs) + q pair-0 s-half-0 --
            k00 = ps_sc.tile([P, ST], F32, name="sc")
            k01 = ps_sc.tile([P, ST], F32, name="sc")
            k10 = ps_av.tile([P, ST], F32, name="av")
            k11 = [ps_px.tile([P, 512], F32, name="px") for _ in range(2)]

            def kacc(pair, th, hf):
                if pair == 0:
                    t = (k00, k01)[th]
                    return t[:, hf * 512 : (hf + 1) * 512]
                if th == 0:
                    return k10[:, hf * 512 : (hf + 1) * 512]
                return k11[hf][:, :]

            for mc in range(MC):
                for pair in range(2):
                    for th in range(2):
                        for hf in range(2):
                            nc.tensor.matmul(
                                kacc(pair, th, hf),
                                wk_sb[:, mc, pair * P : (pair + 1) * P],
                                kg[mc // 2][
                                    :, mc % 2,
                                    th * ST + hf * 512 : th * ST + (hf + 1) * 512,
                                ],
                                start=(mc == 0),
                                stop=(mc == 7),
                            )
            nc.vector.tensor_scalar(
                out=kT[:, 0, 0:ST], in0=k00[:], scalar1=bk_v[:, 0:1],
                scalar2=None, op0=OP.add,
            )
            nc.vector.tensor_scalar(
                out=kT[:, 0, ST : 2 * ST], in0=k01[:], scalar1=bk_v[:, 0:1],
                scalar2=None, op0=OP.add,
            )
            nc.vector.tensor_scalar(
                out=kT[:, 1, 0:ST], in0=k10[:], scalar1=bk_v[:, 1:2],
                scalar2=None, op0=OP.add,
            )
            for hf in range(2):
                nc.vector.tensor_scalar(
                    out=kT[:, 1, ST + hf * 512 : ST + (hf + 1) * 512],
                    in0=k11[hf][:], scalar1=bk_v[:, 1:2],
                    scalar2=None, op0=OP.add,
                )

            # q pair-0, s-half-0 (one [P,1024] accumulator from ps_sc)
            q00 = ps_sc.tile([P, ST], F32, name="sc")
            for mc in range(MC):
                for hf in range(2):
                    nc.tensor.matmul(
                        q00[:, hf * 512 : (hf + 1) * 512],
                        wq_sb[:, mc, 0:P],
                        xqg[(0, mc // 4)][:, mc % 4, hf * 512 : (hf + 1) * 512],
                        start=(mc == 0),
                        stop=(mc == 7),
                    )
            for hf in range(2):
                nc.vector.tensor_scalar(
                    out=qT[:, 0, hf * 512 : (hf + 1) * 512],
                    in0=q00[:, hf * 512 : (hf + 1) * 512],
                    scalar1=bq_v[:, 0:1], scalar2=None, op0=OP.add,
                )

            # ---------------- conveyor job machinery -----------------------
            jobs = []
            vh_done_tc = {}
            norm_state = {}  # block -> step when its norm job fully emitted

            def vh_job(tcc):
                hlf = tcc // 8
                ps = [None]

                def emit(mcs, first):
                    if first:
                        ps[0] = ps_px.tile([P, 512], F32, name="px")
                    for mc in mcs:
                        nc.tensor.matmul(
                            ps[0][:, 0:DH],
                            vt[hlf][:, mc, (tcc % 8) * P : (tcc % 8 + 1) * P],
                            wv_v[:, mc, :],
                            start=(mc == 0),
                            stop=(mc == 7),
                        )
                    if mcs[-1] == 7:
                        nc.vector.tensor_tensor(
                            out=vh[:, tcc, :, 0:DK],
                            in0=ps[0][:, 0:DH].rearrange("p (h d) -> p h d", h=H),
                            in1=bv_v[:].rearrange("p (h d) -> p h d", h=H),
                            op=OP.add,
                        )
                        vh_done_tc[tcc] = True

                return dict(
                    gate=8 if hlf == 0 else 9,
                    batches=[
                        (1024, lambda: emit([0, 1, 2, 3], True)),
                        (1024, lambda: emit([4, 5, 6, 7], False)),
                    ],
                )

            def qproj_job(pair, st2, hf):
                ps = [None]

                def emit(mcs, first):
                    if first:
                        ps[0] = ps_px.tile([P, 512], F32, name="px")
                    for mc in mcs:
                        nc.tensor.matmul(
                            ps[0][:, :],
                            wq_sb[:, mc, pair * P : (pair + 1) * P],
                            xqg[(st2, mc // 4)][:, mc % 4, hf * 512 : (hf + 1) * 512],
                            start=(mc == 0),
                            stop=(mc == 7),
                        )
                    if mcs[-1] == 7:
                        nc.vector.tensor_scalar(
                            out=qT[:, pair, st2 * ST + hf * 512 : st2 * ST + (hf + 1) * 512],
                            in0=ps[0][:, :], scalar1=bq_v[:, pair : pair + 1],
                            scalar2=None, op0=OP.add,
                        )

                return dict(
                    gate=0 if st2 == 0 else 16,
                    batches=[
                        (1024, lambda: emit([0, 1], True)),
                        (1024, lambda: emit([2, 3], False)),
                        (1024, lambda: emit([4, 5], False)),
                        (1024, lambda: emit([6, 7], False)),
                    ],
                )

            def oproj_job(st7, tail=False, gate=102):
                of = [None]

                def emit(nh):
                    if nh == 0:
                        of[0] = op.tile([P, DM], BF16, name="of")
                    ps = ps_px.tile([P, 512], F32, name="px")
                    for c in range(2):
                        nc.tensor.matmul(
                            ps[:, :],
                            (o2a, o2b)[c][:, st7 * P : (st7 + 1) * P],
                            wo_v[:, c, nh * 512 : (nh + 1) * 512],
                            start=(c == 0),
                            stop=(c == 1),
                        )
                    sl = slice(nh * 512, (nh + 1) * 512)
                    if tail and nh == 1:
                        nc.scalar.copy(of[0][:, sl], ps[:, :])
                    else:
                        nc.vector.tensor_copy(of[0][:, sl], ps[:, :])
                    if nh == 1:
                        eng = nc.scalar if (tail and st7 % 2) else nc.sync
                        eng.dma_start(out[st7 * P : (st7 + 1) * P, :], of[0][:])

                return dict(
                    gate=gate,
                    batches=[(512, lambda: emit(0)), (512, lambda: emit(1))],
                )

            def emit_norm(b, avt, step, tail=False):
                # denominator row -> DVE reciprocal -> SBUF->SBUF broadcast
                # DMA (sync ring, idle mid-conveyor) -> DVE multiply straight
                # out of PSUM into o2h.  No PE work, no psum bank use.
                h, st2 = ORDER[b]
                rows = slice(DK * (h % 2), DK * (h % 2) + DK)
                o2h = (o2a, o2b)[h // 2]
                dsb = xp.tile([1, ST], F32, name="nt")
                rsb = xp.tile([1, ST], F32, name="nt")
                scr = xp.tile([1, ST], F32, name="nt")
                if tail:
                    nc.scalar.copy(dsb[0:1, :], avt[DK : DK + 1, :])
                else:
                    nc.vector.tensor_copy(dsb[0:1, :], avt[DK : DK + 1, :])
                nc.vector.reciprocal_approx_accurate(
                    rsb[0:1, :], dsb[0:1, :], scr[0:1, :]
                )
                rdr = dp.tile([1, ST], F32, name="rdr")
                nc.sync.dma_start(rdr[0:1, :], rsb[0:1, :])
                rb = xp.tile([P, ST], F32, name="nt")
                nc.sync.dma_start(rb[rows, :], rdr[0:1, :].to_broadcast((DK, ST)))
                nc.vector.tensor_tensor(
                    out=o2h[rows, st2 * ST : (st2 + 1) * ST],
                    in0=avt[0:DK, :], in1=rb[rows, :], op=OP.mult,
                )
                norm_state[b] = step

            for tcc in range(TC):
                jobs.append(vh_job(tcc))
            jobs.append(qproj_job(1, 0, 0))
            jobs.append(qproj_job(1, 0, 1))
            jobs.append(qproj_job(0, 1, 0))
            jobs.append(qproj_job(0, 1, 1))
            jobs.append(qproj_job(1, 1, 0))
            jobs.append(qproj_job(1, 1, 1))
            for st7 in range(OPROJ_CONVEYOR):
                jobs.append(oproj_job(st7))

            # ---------------- conveyor -------------------------------------
            state = dict(vpe=0, budget=0, active=None, avq=[], curav=None)

            def emit_av(ent, step):
                b, h, tcc, at = ent
                if tcc == 0:
                    state["curav"] = ps_av.tile([P, ST], F32, name="av")
                avt = state["curav"]
                for hf in range(2):
                    nc.tensor.matmul(
                        avt[0 : DK + 1, hf * 512 : (hf + 1) * 512],
                        vh[:, tcc, h, :],
                        at[:, hf * 512 : (hf + 1) * 512],
                        start=(tcc == 0),
                        stop=(tcc == TC - 1),
                    )
                state["vpe"] += 1024
                if tcc == TC - 1:
                    emit_norm(b, avt, step, tail=(b == len(ORDER) - 1))

            def drain(step, budget_cap=True):
                while state["avq"]:
                    b, h, tcc, at = state["avq"][0]
                    if tcc not in vh_done_tc:
                        break
                    if tcc == 0 and b > 0:
                        # previous block's normalize chain must have had time
                        # to drain (its mult frees the single AV accumulator)
                        ns = norm_state.get(b - 1)
                        if ns is None or step < ns + 4:
                            break
                    if budget_cap and state["vpe"] + 1024 > state["budget"]:
                        return
                    emit_av(state["avq"].pop(0), step)
                while True:
                    if state["active"] is None:
                        for i, j in enumerate(jobs):
                            if j["gate"] <= step:
                                state["active"] = jobs.pop(i)
                                break
                        if state["active"] is None:
                            return
                    j = state["active"]
                    cost, fn = j["batches"][0]
                    if budget_cap and state["vpe"] + cost > state["budget"]:
                        return
                    j["batches"].pop(0)
                    fn()
                    state["vpe"] += cost
                    if not j["batches"]:
                        state["active"] = None

            for b, (h, st2) in enumerate(ORDER):
                pair = h // 2
                rows = slice(DK * (h % 2), DK * (h % 2) + DK)
                for tcc in range(TC):
                    step = b * TC + tcc
                    state["budget"] += CYC_PER_STEP
                    sc = ps_sc.tile([P, ST], F32, name="sc")
                    for hf in range(2):
                        nc.tensor.matmul(
                            sc[:, hf * 512 : (hf + 1) * 512],
                            kT[rows, pair, tcc * P : (tcc + 1) * P],
                            qT[rows, pair, st2 * ST + hf * 512 : st2 * ST + (hf + 1) * 512],
                            start=True,
                            stop=True,
                            tile_position=(DK * (h % 2), 0),
                        )
                    state["vpe"] += 1024
                    at = hp.tile([P, ST], BF16, name="at")
                    nc.scalar.activation(at[:], sc[:], AF.Exp)
                    state["avq"].append((b, h, tcc, at))
                    drain(step)

            # ---------------- tail -----------------------------------------
            drain(10**6, budget_cap=False)
            for st7 in range(OPROJ_CONVEYOR, TC):
                j = oproj_job(st7, tail=True)
                for cost, fn in j["batches"]:
                    fn()

    nc.compile()
    return nc


def _get_nc():
    global _COMPILED
    if _COMPILED is None:
        _COMPILED = _build()
    return _COMPILED


def _bf16(x):
    import ml_dtypes

    return np.ascontiguousarray(x.astype(ml_dtypes.bfloat16))


def _make_in_maps(q, k, v, w_q, b_q, w_k, b_k, w_v, b_v, w_o, b_o):
    q = np.asarray(q, np.float32)
    k = np.asarray(k, np.float32)
    v = np.asarray(v, np.float32)
    xs = {}
    for t, arr in (("q", q), ("k", k), ("v", v)):
        for b in range(2):
            xs[(t, b)] = _bf16(np.ascontiguousarray(arr[b].T))
    # fold the 1/sqrt(d_k) score scale into the q projection so the exp
    # activation runs with scale=1
    ws = {"q": np.asarray(w_q, np.float32) * 0.125,
          "k": np.asarray(w_k, np.float32),
          "v": np.asarray(w_v, np.float32)}
    bs = {"q": np.asarray(b_q, np.float32) * 0.125,
          "k": np.asarray(b_k, np.float32),
          "v": np.asarray(b_v, np.float32)}
    w_o = np.asarray(w_o, np.float32)
    in_maps = []
    for core in range(N_CORES):
        b, hg = divmod(core, 4)
        sl = slice(hg * DH, (hg + 1) * DH)
        m = {}
        wpk = {}
        bpk = {}
        for t in ("q", "k", "v"):
            m[f"x{t}"] = xs[(t, b)]
            # pack w.T [DM, DH] as [p, mc*DH]: row p holds chunks mc.
            wT = ws[t][sl, :].T.reshape(MC, P, DH).transpose(1, 0, 2)
            wpk[t] = _bf16(wT.reshape(P, MC * DH))
            bsl = bs[t][sl]
            if t == "v":
                bpk[t] = np.tile(bsl[None, :], (P, 1)).astype(np.float32)
            else:
                bpk[t] = bsl.reshape(2, P).T.astype(np.float32)
        m["wk"] = wpk["k"]
        m["wq"] = wpk["q"]
        woT = w_o[:, sl].T.reshape(2, P, DM).transpose(1, 0, 2)
        m["wvo"] = _bf16(
            np.concatenate([wpk["v"], woT.reshape(P, 2 * DM)], axis=1)
        )
        m["ball"] = np.ascontiguousarray(
            np.concatenate([bpk["q"], bpk["k"], bpk["v"]], axis=1).astype(np.float32)
        )
        in_maps.append(m)
    return in_maps


def run(inputs, trace=False):
    from concourse.bass_utils import run_bass_kernel_spmd

    nc = _get_nc()
    in_maps = _make_in_maps(**inputs)
    res = run_bass_kernel_spmd(
        nc, in_maps, core_ids=list(range(N_CORES)), trace=trace
    )
    b_o = np.asarray(inputs["b_o"], np.float32)
    full = np.empty((2, S, DM), np.float32)
    for b in range(2):
        acc = res.results[4 * b]["out"].astype(np.float32)
        for hg in range(1, 4):
            acc = acc + res.results[4 * b + hg]["out"].astype(np.float32)
        full[b] = acc + b_o[None, :]
    return full, res


def kernel(**inputs) -> np.ndarray:
    full, _ = run(inputs, trace=False)
    return full
